# revision 6
# baseline (speedup 1.0000x reference)
"""Trainium2 Bass kernel for nn_Decoder (2-layer LSTM decoder with Luong
local-p attention + 32k-vocab projection), distributed over 8 NeuronCores.

Sharding:
  - LSTM recurrence: replicated (weight-stream bound); input-gate bulk
    matmuls sharded over (batch,time) / AllGathered.
  - Attention: batch-sharded (4 batches/core), windowed softmax done over
    all 229 encoder positions with data-dependent masks (no gather).
  - fc1: replicated.  fc2 + output: vocab-sharded (4000 rows/core).
Layout: flattened (b, t) index j = b*64 + t everywhere ("b-major").
"""
import sys

sys.path.insert(0, "/opt/trn_rl_repo")

import numpy as np
import ml_dtypes

import concourse.bass as bass
import concourse.mybir as mybir
import concourse.tile as tile
import concourse.bacc as bacc
import concourse.masks as masks
from concourse.bass_utils import run_bass_kernel_spmd

BF16NP = ml_dtypes.bfloat16
F32 = mybir.dt.float32
F32R = mybir.dt.float32r
BF = mybir.dt.bfloat16
I32 = mybir.dt.int32

H = 1024
HH = 512
V = 32000
T = 64
B = 32
WSZ = 50
WLEN = 101
S = 229
NC = 8
BL = B // NC            # local batches per core = 4
TBL = BL * T            # local (b,t) slab = 256
VSL = V // NC           # vocab slice = 4000
VSLP = 4096             # padded vocab slice
TB = T * B              # 2048
AFT = mybir.ActivationFunctionType
ALU = mybir.AluOpType
R2_23 = 8388608.0

_CACHE = {}


def _build_program():
    nc = bacc.Bacc("TRN2", target_bir_lowering=False, debug=False, num_devices=NC)

    P = nc.declare_dram_parameter
    xemb = P("xemb", [H, TBL], BF, isOutput=False)
    xctx = P("xctx", [H, TBL], BF, isOutput=False)
    wih0T = P("wih0T", [2 * H, 4 * H], BF, isOutput=False)
    wih1T = P("wih1T", [H, 4 * H], BF, isOutput=False)
    bias0 = P("bias0", [1, 4 * H], BF, isOutput=False)
    bias1 = P("bias1", [1, 4 * H], BF, isOutput=False)
    whh0T = P("whh0T", [H, 4 * H], F32R, isOutput=False)
    whh1T = P("whh1T", [H, 4 * H], F32R, isOutput=False)
    h0fm = P("h0fm", [2, H, B], F32R, isOutput=False)
    c0bm = P("c0bm", [2, B, H], F32, isOutput=False)
    encfm = P("encfm", [H, BL, S], BF, isOutput=False)
    encbm = P("encbm", [BL, S, H], BF, isOutput=False)
    aw1T = P("aw1T", [H, HH], BF, isOutput=False)
    ab1 = P("ab1", [HH, 1], F32, isOutput=False)
    aw2T = P("aw2T", [HH, 1], BF, isOutput=False)
    ab2 = P("ab2", [1, 1], F32, isOutput=False)
    lensrep = P("lensrep", [1, TB], F32, isOutput=False)
    len64 = P("len64", [T, BL], F32, isOutput=False)
    fc1wT = P("fc1wT", [2 * H, H], BF, isOutput=False)
    fc1b = P("fc1b", [H, 1], F32, isOutput=False)
    fc2wT = P("fc2wT", [H, VSLP], F32R, isOutput=False)
    fc2b = P("fc2b", [VSLP, 1], F32, isOutput=False)

    y_out = P("y_out", [VSLP, TB], BF, isOutput=True)
    ctx_part = P("ctx_part", [BL, H, T], F32, isOutput=True)
    hn_fm = P("hn_fm", [2, H, B], F32R, isOutput=True)
    cn_out = P("cn_out", [2, B, H], F32, isOutput=True)

    ih0_piece = nc.dram_tensor("ih0_piece", [TBL, 4 * H], BF)
    ih0_full = nc.dram_tensor("ih0_full", [TB, 4 * H], BF, addr_space="Shared")
    ih1_piece = nc.dram_tensor("ih1_piece", [TBL, 4 * H], BF)
    ih1_full = nc.dram_tensor("ih1_full", [TB, 4 * H], BF, addr_space="Shared")
    ctxp = nc.dram_tensor("ctxp", [BL, H, T], BF)
    ctx_full = nc.dram_tensor("ctx_full", [B, H, T], BF, addr_space="Shared")

    core_ids = list(range(NC))

    with tile.TileContext(nc) as tc:
        from contextlib import ExitStack

        g = ExitStack()
        gpool = g.enter_context(tc.tile_pool(name="gmisc", bufs=1))
        ident = gpool.tile([128, 128], F32)
        masks.make_identity(nc, ident[:])
        iot = gpool.tile([T, S], I32)
        nc.gpsimd.iota(iot[:], pattern=[[1, S]], base=0, channel_multiplier=0)
        iota_f = gpool.tile([T, S], F32)
        nc.vector.tensor_copy(iota_f[:], iot[:])
        ones1 = gpool.tile([1, 128], BF)
        nc.vector.memset(ones1[:], 1.0)
        ones64f = gpool.tile([1, T], F32)
        nc.vector.memset(ones64f[:], 1.0)

        pid = nc.vector.partition_id()

        # ---------------- Phase A: bulk input gates, layer 0 ----------------
        with ExitStack() as pa:
            apool = pa.enter_context(tc.tile_pool(name="phA", bufs=1))
            apsum = pa.enter_context(tc.tile_pool(name="phA_ps", bufs=4, space="PSUM"))
            xf = []
            for k in range(16):
                xt = apool.tile([128, TBL], BF, name=f"xf{k}")
                src = xemb if k < 8 else xctx
                nc.sync.dma_start(out=xt[:], in_=src[(k % 8) * 128:(k % 8 + 1) * 128, :])
                xf.append(xt)
            w0sb = []
            for k in range(16):
                wt = apool.tile([128, 4 * H], BF, name=f"w0sb{k}")
                nc.sync.dma_start(out=wt[:], in_=wih0T[k * 128:(k + 1) * 128, :])
                w0sb.append(wt)
            b0sb = apool.tile([1, 4 * H], BF)
            nc.sync.dma_start(out=b0sb[:], in_=bias0[:])
            for m in range(2):
                for n in range(8):
                    ps = apsum.tile([128, 512], F32, name="aps", tag="aps", bufs=4)
                    for k in range(16):
                        nc.tensor.matmul(ps[:], xf[k][:, m * 128:(m + 1) * 128],
                                         w0sb[k][:, n * 512:(n + 1) * 512],
                                         start=(k == 0), stop=False)
                    nc.tensor.matmul(ps[:], ones1[0:1, :],
                                     b0sb[0:1, n * 512:(n + 1) * 512],
                                     start=False, stop=True)
                    cp = apool.tile([128, 512], BF, name="acp", tag="acp", bufs=3)
                    nc.vector.tensor_copy(cp[:], ps[:])
                    nc.sync.dma_start(
                        out=ih0_piece[m * 128:(m + 1) * 128, n * 512:(n + 1) * 512],
                        in_=cp[:])
            nc.gpsimd.collective_compute(
                "AllGather", ALU.bypass, ins=[ih0_piece[:]], outs=[ih0_full[:]],
                replica_groups=[core_ids])

        # ---------------- recurrence helper ----------------
        def recurrence(l, ih_full, whh_param, arch, rstack):
            rpool = rstack.enter_context(tc.tile_pool(name=f"rec{l}", bufs=1))
            gps = rstack.enter_context(
                tc.tile_pool(name=f"rec{l}_gps", bufs=4, space="PSUM"))
            tps = rstack.enter_context(
                tc.tile_pool(name=f"rec{l}_tps", bufs=2, space="PSUM"))
            whh_sb = []
            for k in range(8):
                wt = rpool.tile([128, 4 * H], F32R, name=f"whh{l}_{k}")
                nc.sync.dma_start(out=wt[:], in_=whh_param[k * 128:(k + 1) * 128, :])
                whh_sb.append(wt)
            hcur = []
            for k in range(8):
                ht = rpool.tile([128, B], F32R, name=f"h{l}init{k}",
                                tag=f"hfm{l}", bufs=16)
                nc.sync.dma_start(out=ht[:], in_=h0fm[l, k * 128:(k + 1) * 128, :])
                hcur.append(ht)
            cprev = rpool.tile([2 * B, H], F32, name=f"c{l}init", tag=f"c{l}", bufs=2)
            nc.sync.dma_start(out=cprev[32:64, :], in_=c0bm[l])
            ih3 = ih_full[:].rearrange("(b t) g -> b t g", t=T)
            arch3 = [a[:].rearrange("p (b t) -> p b t", t=T) for a in arch]

            for t in range(T):
                ihh = []
                for hf in range(2):
                    it_ = rpool.tile([B, 2 * H], BF, name=f"ih{l}_{t}_{hf}",
                                     tag=f"ih{l}{hf}", bufs=2)
                    nc.sync.dma_start(out=it_[:], in_=ih3[:, t, hf * 2048:(hf + 1) * 2048])
                    ihh.append(it_)
                # tileX: sig_i @[0:32], sig_f @[32:64]; tileY: tanh_g @[0:32], sig_o @[32:64]
                tileX = rpool.tile([2 * B, H], F32, name=f"tX{l}_{t}", tag=f"tX{l}", bufs=1)
                tileY = rpool.tile([2 * B, H], F32, name=f"tY{l}_{t}", tag=f"tY{l}", bufs=1)
                for n in range(8):
                    gp = gps.tile([B, 512], F32, name=f"gp{l}_{t}_{n}",
                                  tag=f"gp{l}", bufs=4)
                    for k in range(8):
                        nc.tensor.matmul(gp[:], hcur[k][:],
                                         whh_sb[k][:, n * 512:(n + 1) * 512],
                                         start=(k == 0), stop=(k == 7))
                    blk, half = divmod(n, 2)
                    dstt = tileX if blk < 2 else tileY
                    roff = 32 * (blk % 2)
                    dst = dstt[roff:roff + 32, 512 * half:512 * half + 512]
                    nc.vector.tensor_tensor(
                        dst, gp[:], ihh[n // 4][:, (n % 4) * 512:(n % 4 + 1) * 512],
                        op=ALU.add)
                nc.scalar.activation(tileX[0:32, :], tileX[0:32, :], AFT.Sigmoid)
                nc.scalar.activation(tileX[32:64, :], tileX[32:64, :], AFT.Sigmoid)
                nc.scalar.activation(tileY[0:32, :], tileY[0:32, :], AFT.Tanh)
                nc.scalar.activation(tileY[32:64, :], tileY[32:64, :], AFT.Sigmoid)
                tmpA = rpool.tile([B, H], F32, name=f"tA{l}_{t}", tag=f"tA{l}", bufs=1)
                tmpB = rpool.tile([B, H], F32, name=f"tB{l}_{t}", tag=f"tB{l}", bufs=1)
                cnew = rpool.tile([2 * B, H], F32, name=f"c{l}_{t}", tag=f"c{l}", bufs=2)
                # t2 = sig_i * tanh_g ; t1 = sig_f * c ; c' = t1 + t2
                nc.vector.tensor_tensor(tmpA[:], tileX[0:32, :], tileY[0:32, :],
                                        op=ALU.mult)
                nc.vector.tensor_tensor(tmpB[:], tileX[32:64, :], cprev[32:64, :],
                                        op=ALU.mult)
                nc.vector.tensor_tensor(cnew[32:64, :], tmpA[:], tmpB[:], op=ALU.add)
                # tanh(c') -> tileX[32:64]; h = sig_o * tanh(c') -> tmpA
                nc.scalar.activation(tileX[32:64, :], cnew[32:64, :], AFT.Tanh)
                nc.vector.tensor_tensor(tmpA[:], tileY[32:64, :], tileX[32:64, :],
                                        op=ALU.mult)
                hnew = []
                for k in range(8):
                    tp = tps.tile([128, B], F32, name=f"tp{l}_{t}_{k}",
                                  tag=f"tp{l}", bufs=2)
                    nc.tensor.transpose(tp[:], tmpA[:, k * 128:(k + 1) * 128],
                                        ident[0:32, 0:32])
                    hk = rpool.tile([128, B], F32R, name=f"h{l}_{t}_{k}",
                                    tag=f"hfm{l}", bufs=16)
                    nc.vector.tensor_copy(hk[:], tp[:])
                    nc.vector.tensor_copy(arch3[k][:, :, t], tp[:])
                    hnew.append(hk)
                hcur = hnew
                cprev = cnew
            for k in range(8):
                nc.sync.dma_start(out=hn_fm[l, k * 128:(k + 1) * 128, :],
                                  in_=hcur[k][:])
            nc.sync.dma_start(out=cn_out[l], in_=cprev[32:64, :])

        # ---------------- Phase B: layer-0 recurrence ----------------
        y0stack = ExitStack()
        y0pool = y0stack.enter_context(tc.tile_pool(name="y0arch", bufs=1))
        y0arch = [y0pool.tile([128, TB], BF, name=f"y0a{k}") for k in range(8)]
        with ExitStack() as pb:
            recurrence(0, ih0_full, whh0T, y0arch, pb)

        # ---------------- Phase C: bulk input gates, layer 1 ----------------
        with ExitStack() as pc:
            cpool = pc.enter_context(tc.tile_pool(name="phC", bufs=1))
            cpsum = pc.enter_context(tc.tile_pool(name="phC_ps", bufs=4, space="PSUM"))
            slab = []
            for k in range(8):
                st = cpool.tile([128, TBL], BF, name=f"slab{k}")
                nc.vector.tensor_copy(st[:], y0arch[k][:, bass.ds(pid * TBL, TBL)])
                slab.append(st)
            w1sb = []
            for k in range(8):
                wt = cpool.tile([128, 4 * H], BF, name=f"w1sb{k}")
                nc.sync.dma_start(out=wt[:], in_=wih1T[k * 128:(k + 1) * 128, :])
                w1sb.append(wt)
            b1sb = cpool.tile([1, 4 * H], BF)
            nc.sync.dma_start(out=b1sb[:], in_=bias1[:])
            for m in range(2):
                for n in range(8):
                    ps = cpsum.tile([128, 512], F32, name="cps", tag="cps", bufs=4)
                    for k in range(8):
                        nc.tensor.matmul(ps[:], slab[k][:, m * 128:(m + 1) * 128],
                                         w1sb[k][:, n * 512:(n + 1) * 512],
                                         start=(k == 0), stop=False)
                    nc.tensor.matmul(ps[:], ones1[0:1, :],
                                     b1sb[0:1, n * 512:(n + 1) * 512],
                                     start=False, stop=True)
                    cp = cpool.tile([128, 512], BF, name="ccp", tag="ccp", bufs=3)
                    nc.vector.tensor_copy(cp[:], ps[:])
                    nc.sync.dma_start(
                        out=ih1_piece[m * 128:(m + 1) * 128, n * 512:(n + 1) * 512],
                        in_=cp[:])
            nc.gpsimd.collective_compute(
                "AllGather", ALU.bypass, ins=[ih1_piece[:]], outs=[ih1_full[:]],
                replica_groups=[core_ids])

        y0stack.close()

        # ---------------- Phase D: layer-1 recurrence ----------------
        with ExitStack() as pd:
            y1pool = g.enter_context(tc.tile_pool(name="y1arch", bufs=1))
            y1arch = [y1pool.tile([128, TB], BF, name=f"y1a{k}") for k in range(8)]
            recurrence(1, ih1_full, whh1T, y1arch, pd)

        # ---------------- Phase E: attention ----------------
        with ExitStack() as pe:
            epool = pe.enter_context(tc.tile_pool(name="phE", bufs=1))
            peP = ExitStack()
            eps_p = peP.enter_context(tc.tile_pool(name="phE_psp", bufs=2, space="PSUM"))
            # p-path
            aw1sb = []
            for k in range(8):
                at = epool.tile([128, HH], BF, name=f"aw1sb{k}")
                nc.sync.dma_start(out=at[:], in_=aw1T[k * 128:(k + 1) * 128, :])
                aw1sb.append(at)
            ab1sb = []
            for m in range(4):
                bt = epool.tile([128, 1], F32, name=f"ab1sb{m}")
                nc.sync.dma_start(out=bt[:], in_=ab1[m * 128:(m + 1) * 128, :])
                ab1sb.append(bt)
            aw2sb = epool.tile([128, 4], BF)
            nc.sync.dma_start(out=aw2sb[:],
                              in_=aw2T[:].rearrange("(i p) c -> p (i c)", p=128))
            ab2sb = epool.tile([1, 1], F32)
            nc.sync.dma_start(out=ab2sb[:], in_=ab2[:])
            lenssb = epool.tile([1, TB], F32)
            nc.sync.dma_start(out=lenssb[:], in_=lensrep[:])
            len64sb = epool.tile([T, BL], F32)
            nc.sync.dma_start(out=len64sb[:], in_=len64[:])

            a1sb = [epool.tile([128, TB], BF, name=f"a1sb{m}") for m in range(4)]
            for m in range(4):
                for n in range(4):
                    ps = eps_p.tile([128, 512], F32, name="a1ps", tag="a1ps", bufs=2)
                    for k in range(8):
                        nc.tensor.matmul(ps[:], aw1sb[k][:, m * 128:(m + 1) * 128],
                                         y1arch[k][:, n * 512:(n + 1) * 512],
                                         start=(k == 0), stop=(k == 7))
                    nc.scalar.activation(a1sb[m][:, n * 512:(n + 1) * 512], ps[:],
                                         AFT.Tanh, bias=ab1sb[m][:])
            psig = epool.tile([1, TB], F32)
            for n in range(4):
                pps = eps_p.tile([1, 512], F32, name="pps", tag="pps", bufs=2)
                for k2 in range(4):
                    nc.tensor.matmul(pps[:],
                                     aw2sb[:, k2:k2 + 1],
                                     a1sb[k2][:, n * 512:(n + 1) * 512],
                                     start=(k2 == 0), stop=(k2 == 3))
                nc.scalar.activation(psig[:, n * 512:(n + 1) * 512], pps[:],
                                     AFT.Sigmoid, bias=ab2sb[:])
            q = epool.tile([1, TB], F32)
            nc.vector.tensor_tensor(q[:], psig[:], lenssb[:], op=ALU.mult)
            win0 = epool.tile([1, TB], F32)
            nc.vector.tensor_scalar_add(win0[:], q[:], R2_23)
            nc.vector.tensor_scalar_add(win0[:], win0[:], -R2_23)
            peP.close()
            eps = pe.enter_context(tc.tile_pool(name="phE_psb", bufs=2, space="PSUM"))

            enc0 = []
            enc1 = []
            for bl in range(BL):
                e0 = epool.tile([128, H], BF, name=f"encb0_{bl}")
                nc.sync.dma_start(out=e0[:], in_=encbm[bl, 0:128, :])
                e1 = epool.tile([101, H], BF, name=f"encb1_{bl}")
                nc.sync.dma_start(out=e1[:], in_=encbm[bl, 128:S, :])
                enc0.append(e0)
                enc1.append(e1)
            encf = []
            for k in range(8):
                ef = epool.tile([128, BL * S], BF, name=f"encf{k}")
                nc.sync.dma_start(out=ef[:], in_=encfm[k * 128:(k + 1) * 128, :, :])
                encf.append(ef)

            for bl in range(BL):
                off = (pid * BL + bl) * T
                qrow = epool.tile([1, T], F32, name="qrow", tag="qrow", bufs=2)
                nc.vector.tensor_copy(qrow[:], q[0:1, bass.ds(off, T)])
                wrow = epool.tile([1, T], F32, name="wrow", tag="wrow", bufs=2)
                nc.vector.tensor_copy(wrow[:], win0[0:1, bass.ds(off, T)])
                # transpose to columns
                tq = eps.tile([T, 1], F32, name="tq", tag="ttr", bufs=2)
                nc.tensor.matmul(tq[:], qrow[:], ident[0:1, 0:1], start=True, stop=True)
                qcol = epool.tile([T, 1], F32, name="qcol", tag="qcol", bufs=2)
                nc.vector.tensor_copy(qcol[:], tq[:])
                tw_ = eps.tile([T, 1], F32, name="tw", tag="ttr", bufs=2)
                nc.tensor.matmul(tw_[:], wrow[:], ident[0:1, 0:1], start=True, stop=True)
                wcol = epool.tile([T, 1], F32, name="wcol", tag="wcol", bufs=2)
                nc.vector.tensor_copy(wcol[:], tw_[:])
                tb_ = eps.tile([T, 1], F32, name="tb", tag="ttr", bufs=2)
                nc.tensor.matmul(tb_[:], ones64f[0:1, :], wrow[0:1, 63:64],
                                 start=True, stop=True)
                w63col = epool.tile([T, 1], F32, name="w63col", tag="w63", bufs=2)
                nc.vector.tensor_copy(w63col[:], tb_[:])
                # bounds
                bLo = epool.tile([T, 1], F32, name="bLo", tag="bLo", bufs=2)
                nc.vector.tensor_tensor(bLo[:], w63col[:], wcol[:], op=ALU.subtract)
                nc.vector.tensor_scalar_add(bLo[:], bLo[:], float(WSZ))
                bHi = epool.tile([T, 1], F32, name="bHi", tag="bHi", bufs=2)
                nc.vector.tensor_tensor(bHi[:], bLo[:], len64sb[:, bl:bl + 1],
                                        op=ALU.add)
                aHi = epool.tile([T, 1], F32, name="aHi", tag="aHi", bufs=2)
                nc.vector.tensor_scalar_add(aHi[:], w63col[:], float(WLEN))
                geff = epool.tile([T, 1], F32, name="geff", tag="geff", bufs=2)
                nc.vector.tensor_tensor(geff[:], qcol[:], wcol[:], op=ALU.subtract)
                nc.vector.tensor_tensor(geff[:], geff[:], w63col[:], op=ALU.add)
                nc.vector.tensor_scalar_add(geff[:], geff[:], float(WSZ))
                # extract y1_b tiles
                y1b = []
                for k in range(8):
                    yt = epool.tile([128, T], BF, name=f"y1b{k}", tag=f"y1b{k}",
                                    bufs=2)
                    nc.vector.tensor_copy(yt[:], y1arch[k][:, bass.ds(off, T)])
                    y1b.append(yt)
                scps = eps.tile([T, S], F32, name="scps", tag="scps", bufs=2)
                for k in range(8):
                    nc.tensor.matmul(scps[:], y1b[k][:],
                                     encf[k][:, bl * S:(bl + 1) * S],
                                     start=(k == 0), stop=(k == 7))
                # masks
                def cmp_tile(nm, op, bound):
                    mt = epool.tile([T, S], F32, name=nm, tag=nm, bufs=2)
                    nc.vector.tensor_scalar(mt[:], iota_f[:], bound[:], None, op0=op)
                    return mt
                geA = cmp_tile("geA", ALU.is_ge, w63col)
                ltA = cmp_tile("ltA", ALU.is_lt, aHi)
                geB = cmp_tile("geB", ALU.is_ge, bLo)
                ltB = cmp_tile("ltB", ALU.is_lt, bHi)
                inA = epool.tile([T, S], F32, name="inA", tag="inA", bufs=2)
                nc.vector.tensor_tensor(inA[:], geA[:], ltA[:], op=ALU.mult)
                inB = epool.tile([T, S], F32, name="inB", tag="inB", bufs=2)
                nc.vector.tensor_tensor(inB[:], geB[:], ltB[:], op=ALU.mult)
                seff = epool.tile([T, S], F32, name="seff", tag="seff", bufs=2)
                nc.vector.tensor_tensor(seff[:], scps[:], inB[:], op=ALU.mult)
                nc.vector.tensor_tensor(seff[:], seff[:], inA[:], op=ALU.mult)
                pen = epool.tile([T, S], F32, name="pen", tag="pen", bufs=2)
                nc.vector.tensor_scalar(pen[:], inA[:], 100000.0, -100000.0,
                                        op0=ALU.mult, op1=ALU.add)
                nc.vector.tensor_tensor(seff[:], seff[:], pen[:], op=ALU.add)
                # softmax
                mx = epool.tile([T, 1], F32, name="mx", tag="mx", bufs=2)
                nc.vector.reduce_max(mx[:], seff[:], axis=mybir.AxisListType.X)
                nmx = epool.tile([T, 1], F32, name="nmx", tag="nmx", bufs=2)
                nc.vector.tensor_scalar_mul(nmx[:], mx[:], -1.0)
                ex = epool.tile([T, S], F32, name="ex", tag="ex", bufs=2)
                nc.scalar.activation(ex[:], seff[:], AFT.Exp, bias=nmx[:])
                dn = epool.tile([T, 1], F32, name="dn", tag="dn", bufs=2)
                nc.vector.reduce_sum(dn[:], ex[:], axis=mybir.AxisListType.X)
                rc = epool.tile([T, 1], F32, name="rc", tag="rc", bufs=2)
                nc.vector.reciprocal(rc[:], dn[:])
                dv = epool.tile([T, S], F32, name="dv", tag="dv", bufs=2)
                nc.vector.tensor_scalar(dv[:], iota_f[:], geff[:], None,
                                        op0=ALU.subtract)
                d2 = epool.tile([T, S], F32, name="d2", tag="d2", bufs=2)
                nc.scalar.activation(d2[:], dv[:], AFT.Square)
                ga = epool.tile([T, S], F32, name="ga", tag="ga", bufs=2)
                nc.scalar.activation(ga[:], d2[:], AFT.Exp,
                                     scale=-1.0 / (2.0 * (WSZ / 2.0) ** 2))
                am = epool.tile([T, S], F32, name="am", tag="am", bufs=2)
                nc.vector.tensor_tensor(am[:], ex[:], ga[:], op=ALU.mult)
                nc.vector.tensor_scalar(am[:], am[:], rc[:], None, op0=ALU.mult)
                # transpose a -> aT (two chunks)
                ta0 = eps.tile([128, T], F32, name="ta0", tag="ta", bufs=2)
                nc.tensor.transpose(ta0[:], am[:, 0:128], ident[0:T, 0:T])
                aT0 = epool.tile([128, T], BF, name="aT0", tag="aT0", bufs=2)
                nc.vector.tensor_copy(aT0[:], ta0[:])
                ta1 = eps.tile([101, T], F32, name="ta1", tag="ta", bufs=2)
                nc.tensor.transpose(ta1[:], am[:, 128:S], ident[0:T, 0:T])
                aT1 = epool.tile([101, T], BF, name="aT1", tag="aT1", bufs=2)
                nc.vector.tensor_copy(aT1[:], ta1[:])
                # ctx
                for m in range(8):
                    cps = eps.tile([128, T], F32, name="cps", tag="ctxps", bufs=2)
                    nc.tensor.matmul(cps[:], enc0[bl][:, m * 128:(m + 1) * 128],
                                     aT0[:], start=True, stop=False)
                    nc.tensor.matmul(cps[:], enc1[bl][:, m * 128:(m + 1) * 128],
                                     aT1[:], start=False, stop=True)
                    cf = epool.tile([128, T], F32, name="cf", tag="cf", bufs=2)
                    nc.vector.tensor_copy(cf[:], cps[:])
                    nc.sync.dma_start(out=ctx_part[bl, m * 128:(m + 1) * 128, :],
                                      in_=cf[:])
                    cb = epool.tile([128, T], BF, name="cb", tag="cb", bufs=2)
                    nc.vector.tensor_copy(cb[:], cps[:])
                    nc.sync.dma_start(out=ctxp[bl, m * 128:(m + 1) * 128, :],
                                      in_=cb[:])
            nc.gpsimd.collective_compute(
                "AllGather", ALU.bypass, ins=[ctxp[:]], outs=[ctx_full[:]],
                replica_groups=[core_ids])

        # ---------------- Phase F: fc1 (replicated) ----------------
        out1 = []
        o1pool = g.enter_context(tc.tile_pool(name="out1", bufs=1))
        with ExitStack() as pf:
            fpool = pf.enter_context(tc.tile_pool(name="phF", bufs=1))
            fps = pf.enter_context(tc.tile_pool(name="phF_ps", bufs=4, space="PSUM"))
            fc1sb = []
            for k in range(16):
                ft = fpool.tile([128, H], BF, name=f"fc1sb{k}")
                nc.sync.dma_start(out=ft[:], in_=fc1wT[k * 128:(k + 1) * 128, :])
                fc1sb.append(ft)
            fc1bsb = []
            for m in range(8):
                bt = fpool.tile([128, 1], F32, name=f"fc1bsb{m}")
                nc.sync.dma_start(out=bt[:], in_=fc1b[m * 128:(m + 1) * 128, :])
                fc1bsb.append(bt)
            ctxfm = []
            for k in range(8):
                ct = fpool.tile([128, TB], BF, name=f"ctxfm{k}")
                nc.sync.dma_start(
                    out=ct[:],
                    in_=ctx_full[:, k * 128:(k + 1) * 128, :].rearrange(
                        "b h t -> h b t"))
                ctxfm.append(ct)
            rhs = ctxfm + y1arch
            for m in range(8):
                o1 = o1pool.tile([128, TB], F32R, name=f"out1_{m}")
                for n in range(4):
                    ps = fps.tile([128, 512], F32, name="fps", tag="fps", bufs=4)
                    for k in range(16):
                        nc.tensor.matmul(ps[:], fc1sb[k][:, m * 128:(m + 1) * 128],
                                         rhs[k][:, n * 512:(n + 1) * 512],
                                         start=(k == 0), stop=(k == 15))
                    nc.scalar.activation(o1[:, n * 512:(n + 1) * 512], ps[:],
                                         AFT.Relu, bias=fc1bsb[m][:])
                out1.append(o1)

        # ---------------- Phase G: fc2 (vocab-sharded) ----------------
        with ExitStack() as pg:
            gfpool = pg.enter_context(tc.tile_pool(name="phG", bufs=1))
            gps_ = pg.enter_context(tc.tile_pool(name="phG_ps", bufs=4, space="PSUM"))
            fc2bsb = gfpool.tile([128, 32], F32)
            nc.sync.dma_start(out=fc2bsb[:],
                              in_=fc2b[:].rearrange("(m p) c -> p (m c)", p=128))
            for m in range(32):
                fw = []
                for k in range(8):
                    wt = gfpool.tile([128, 128], F32R, name=f"fw_{m}_{k}",
                                     tag=f"fw{k}", bufs=3)
                    nc.sync.dma_start(
                        out=wt[:],
                        in_=fc2wT[k * 128:(k + 1) * 128, m * 128:(m + 1) * 128])
                    fw.append(wt)
                ysb = gfpool.tile([128, TB], BF, name=f"ysb{m}", tag="ysb", bufs=2)
                for n in range(4):
                    ps = gps_.tile([128, 512], F32, name="gps", tag="gps", bufs=4)
                    for k in range(8):
                        nc.tensor.matmul(ps[:], fw[k][:],
                                         out1[k][:, n * 512:(n + 1) * 512],
                                         start=(k == 0), stop=(k == 7))
                    nc.scalar.activation(ysb[:, n * 512:(n + 1) * 512], ps[:],
                                         AFT.Identity, bias=fc2bsb[:, m:m + 1])
                nc.sync.dma_start(out=y_out[m * 128:(m + 1) * 128, :], in_=ysb[:])

        g.close()

    nc.compile()
    return nc


def _prep_inputs(inputs):
    f32 = np.float32

    def a(x):
        return np.asarray(x)

    enc = a(inputs["encoder_output"]).astype(f32)       # [S, B, H]
    context = a(inputs["context"]).astype(f32)          # [T, B, H]
    h0 = a(inputs["h0"]).astype(f32)
    c0 = a(inputs["c0"]).astype(f32)
    embedding = a(inputs["embedding"]).astype(f32)
    aw1 = a(inputs["attn_w1"]).astype(f32)
    ab1 = a(inputs["attn_b1"]).astype(f32)
    aw2 = a(inputs["attn_w2"]).astype(f32)
    ab2 = a(inputs["attn_b2"]).astype(f32)
    wih0 = a(inputs["lstm_Wih0"]).astype(f32)
    whh0 = a(inputs["lstm_Whh0"]).astype(f32)
    bih0 = a(inputs["lstm_bih0"]).astype(f32)
    bhh0 = a(inputs["lstm_bhh0"]).astype(f32)
    wih1 = a(inputs["lstm_Wih1"]).astype(f32)
    whh1 = a(inputs["lstm_Whh1"]).astype(f32)
    bih1 = a(inputs["lstm_bih1"]).astype(f32)
    bhh1 = a(inputs["lstm_bhh1"]).astype(f32)
    fc1w = a(inputs["fc1_w"]).astype(f32)
    fc1b = a(inputs["fc1_b"]).astype(f32)
    fc2w = a(inputs["fc2_w"]).astype(f32)
    fc2b = a(inputs["fc2_b"]).astype(f32)
    tw = a(inputs["target_words"]).astype(np.int64)     # [T, B]
    lengths = a(inputs["lengths"]).astype(f32)          # [B]

    emb = embedding[tw]                                 # [T, B, H]

    shared = dict(
        wih0T=np.ascontiguousarray(wih0.T).astype(BF16NP),
        wih1T=np.ascontiguousarray(wih1.T).astype(BF16NP),
        bias0=(bih0 + bhh0)[None, :].astype(BF16NP),
        bias1=(bih1 + bhh1)[None, :].astype(BF16NP),
        whh0T=np.ascontiguousarray(whh0.T),
        whh1T=np.ascontiguousarray(whh1.T),
        h0fm=np.ascontiguousarray(h0.transpose(0, 2, 1)),
        c0bm=np.ascontiguousarray(c0),
        aw1T=np.ascontiguousarray(aw1.T).astype(BF16NP),
        ab1=ab1[:, None].copy(),
        aw2T=np.ascontiguousarray(aw2.T).astype(BF16NP),
        ab2=ab2[:, None].copy(),
        lensrep=np.repeat(lengths, T)[None, :].copy(),
        fc1wT=np.ascontiguousarray(fc1w.T).astype(BF16NP),
        fc1b=fc1b[:, None].copy(),
    )
    in_maps = []
    for c in range(NC):
        bs = slice(BL * c, BL * (c + 1))
        m = dict(shared)
        m["xemb"] = np.ascontiguousarray(
            emb[:, bs, :].transpose(1, 0, 2).reshape(TBL, H).T).astype(BF16NP)
        m["xctx"] = np.ascontiguousarray(
            context[:, bs, :].transpose(1, 0, 2).reshape(TBL, H).T).astype(BF16NP)
        m["encfm"] = np.ascontiguousarray(
            enc[:, bs, :].transpose(2, 1, 0)).astype(BF16NP)
        m["encbm"] = np.ascontiguousarray(
            enc[:, bs, :].transpose(1, 0, 2)).astype(BF16NP)
        m["len64"] = np.broadcast_to(lengths[bs], (T, BL)).copy()
        w2 = np.zeros((H, VSLP), f32)
        w2[:, :VSL] = fc2w[VSL * c:VSL * (c + 1)].T
        m["fc2wT"] = w2
        b2 = np.zeros((VSLP, 1), f32)
        b2[:VSL, 0] = fc2b[VSL * c:VSL * (c + 1)]
        m["fc2b"] = b2
        in_maps.append(m)
    return in_maps


def _get_program():
    if "nc" not in _CACHE:
        _CACHE["nc"] = _build_program()
    return _CACHE["nc"]


def run_on_device(in_maps):
    nc = _get_program()
    res = run_bass_kernel_spmd(nc, in_maps, list(range(NC)))
    return res.results


def assemble_outputs(results):
    f32 = np.float32
    y_all = np.concatenate(
        [results[c]["y_out"][:VSL].astype(f32) for c in range(NC)], axis=0)
    y = np.ascontiguousarray(y_all.reshape(V, B, T).transpose(2, 1, 0))
    ctx_all = np.concatenate(
        [results[c]["ctx_part"].astype(f32) for c in range(NC)], axis=0)
    ctx = np.ascontiguousarray(ctx_all.transpose(2, 0, 1))
    h_n = np.ascontiguousarray(results[0]["hn_fm"].transpose(0, 2, 1)).astype(f32)
    c_n = results[0]["cn_out"].astype(f32)
    return y, h_n, c_n, ctx


def kernel(**inputs):
    in_maps = _prep_inputs(inputs)
    results = run_on_device(in_maps)
    return assemble_outputs(results)
